# revision 5
# baseline (speedup 1.0000x reference)
# Trainium2 Bass kernel for nn_CauRecNet (2-layer residual-cell LSTM scan).
#
# Strategy: pure data-parallel over 8 NeuronCores (batch 131072 -> 16384/core).
# Per core: For_i over 16 "pair tiles" of 1024 batch rows = two 512-halves
# (A, B). Activations are feature-major ([feature, batch] in SBUF) so batch
# rides the matmul free dim (N=512).
#
# The small layer-0 (H1=64) is processed with both halves STACKED on the
# partition axis ([A;B] -> 128 rows) so every pointwise op runs on full
# [128,512] tiles. The stacked gate matmuls use BLOCK-DIAGONAL weights
# (lhsT = [[W,0],[0,W]]) so they stay plain 128x128-mode matmuls.
# Layer-1 (H2=128) runs per half; its ih-matmul reads the stacked h0 via
# half-masked (zero-padded) weights.
#
# dtypes: matmuls bf16, gate activations / intermediates bf16, c-state fp32.
# numpy-simulated rel-l2 error vs fp32 reference: ~5e-3.

import numpy as np
import ml_dtypes

B, T, F = 131072, 15, 12
H1, H2, CS = 64, 128, 96
NCORES = 8
BL = B // NCORES          # 16384 rows per core
NT = 512                  # matmul free dim (one half)
NPAIR = BL // (2 * NT)    # 16 pair-tiles per core

BF16 = ml_dtypes.bfloat16

_BUILD_CACHE = {}


def _build_bass(has_gate_bias, has_vec_bias):
    import concourse.bacc as bacc
    import concourse.tile as tile
    from concourse import mybir
    from concourse.masks import make_identity

    f32 = mybir.dt.float32
    bf16 = mybir.dt.bfloat16
    AF = mybir.ActivationFunctionType

    nc = bacc.Bacc()

    # ---- DRAM I/O ----
    x_d = nc.dram_tensor("input_seq", [BL, T, F], f32, kind="ExternalInput")
    cs_d = nc.dram_tensor("cell_state", [BL, CS], f32, kind="ExternalInput")
    w0ih_d = nc.dram_tensor("w0ih_bd", [2 * F, 4 * H1 * 2], bf16, kind="ExternalInput")
    w0hh_d = nc.dram_tensor("w0hh_bd", [2 * H1, 4 * H1 * 2], bf16, kind="ExternalInput")
    w1ihA_d = nc.dram_tensor("w1ih_A", [2 * H1, 4 * H2], bf16, kind="ExternalInput")
    w1ihB_d = nc.dram_tensor("w1ih_B", [2 * H1, 4 * H2], bf16, kind="ExternalInput")
    w1hh_d = nc.dram_tensor("w1hhT", [H2, 4 * H2], bf16, kind="ExternalInput")
    fc1A_d = nc.dram_tensor("fc1_A", [CS, 2 * H1], f32, kind="ExternalInput")
    fc1B_d = nc.dram_tensor("fc1_B", [CS, 2 * H1], f32, kind="ExternalInput")
    fc2_d = nc.dram_tensor("fc2T", [CS, H2], f32, kind="ExternalInput")
    d1_d = nc.dram_tensor("d1T", [H2, H1], bf16, kind="ExternalInput")
    d2_d = nc.dram_tensor("d2T", [H1, 1], bf16, kind="ExternalInput")
    gb_d = nc.dram_tensor("gate_bias", [128, 8], f32, kind="ExternalInput")
    vb_d = nc.dram_tensor("vec_bias", [128, 4], f32, kind="ExternalInput")
    pred_d = nc.dram_tensor("pred", [BL, 1], f32, kind="ExternalOutput")

    # DRAM views ([pair, ...])
    x_view = x_d[:].rearrange("(n c p) t f -> n p c (t f)", c=8, p=128)    # [16,128,8,180]
    cs_view = cs_d[:].rearrange("(n c p) k -> n p c k", c=8, p=128)        # [16,128,8,96]
    pred_view = pred_d[:].rearrange("(n h x) o -> n h o x", h=2, x=NT)     # [16,2,1,512]

    with tile.TileContext(nc) as tc:
        import contextlib
        ctx = contextlib.ExitStack()
        with ctx:
            consts = ctx.enter_context(tc.tile_pool(name="consts", bufs=1))
            loads = ctx.enter_context(tc.tile_pool(name="loads", bufs=2))
            xts = ctx.enter_context(tc.tile_pool(name="xts", bufs=2))
            states = ctx.enter_context(tc.tile_pool(name="states", bufs=3))
            scratch = ctx.enter_context(tc.tile_pool(name="scratch", bufs=2))
            outp = ctx.enter_context(tc.tile_pool(name="outp", bufs=2))
            pp_g0 = ctx.enter_context(tc.tile_pool(name="pp_g0", bufs=1, space="PSUM"))
            pp_g1 = ctx.enter_context(tc.tile_pool(name="pp_g1", bufs=1, space="PSUM"))

            # ---- constants / weights (loaded once) ----
            ident = consts.tile([128, 128], f32)
            make_identity(nc, ident)

            def load_const(name, dram, shape, dt):
                t = consts.tile(shape, dt, name=name)
                nc.sync.dma_start(out=t, in_=dram[:])
                return t

            w0ih = load_const("w0ih", w0ih_d, [2 * F, 512], bf16)
            w0hh = load_const("w0hh", w0hh_d, [2 * H1, 512], bf16)
            w1ihA = load_const("w1ihA", w1ihA_d, [2 * H1, 512], bf16)
            w1ihB = load_const("w1ihB", w1ihB_d, [2 * H1, 512], bf16)
            w1hh = load_const("w1hh", w1hh_d, [H2, 512], bf16)
            fc1A = load_const("fc1A", fc1A_d, [CS, 128], f32)
            fc1B = load_const("fc1B", fc1B_d, [CS, 128], f32)
            fc2 = load_const("fc2", fc2_d, [CS, H2], f32)
            d1w = load_const("d1w", d1_d, [H2, H1], bf16)
            d2w = load_const("d2w", d2_d, [H1, 1], bf16)
            gbias = load_const("gbias", gb_d, [128, 8], f32)
            vbias = load_const("vbias", vb_d, [128, 4], f32)

            def pair_body(it):
                # ---------- load ----------
                x_nat = loads.tile([128, 8, T * F], f32, tag="x_nat")
                nc.sync.dma_start(out=x_nat, in_=x_view[it])
                cs_nat = loads.tile([128, 8, CS], f32, tag="cs_nat")
                nc.sync.dma_start(out=cs_nat, in_=cs_view[it])

                # ---------- transpose to feature-major (PE) ----------
                tp_x = pp_g1.tile([128, 2048], f32, tag="G1")
                tp_c = pp_g1.tile([128, 2048], f32, tag="G1")
                for c in range(8):
                    nc.tensor.transpose(tp_x[0:96, c * 128:(c + 1) * 128],
                                        x_nat[:, c, 0:96], ident)
                    nc.tensor.transpose(tp_x[0:96, 1024 + c * 128:1024 + (c + 1) * 128],
                                        x_nat[:, c, 84:180], ident)
                    nc.tensor.transpose(tp_c[0:96, c * 128:(c + 1) * 128],
                                        cs_nat[:, c, :], ident)
                xT_lo = xts.tile([96, 1024], bf16, tag="xT_lo")  # feats 0:96, t 0..7
                nc.vector.tensor_copy(out=xT_lo, in_=tp_x[0:96, 0:1024])
                xT_hi = xts.tile([96, 1024], bf16, tag="xT_hi")  # feats 84:180, t 7..14
                nc.vector.tensor_copy(out=xT_hi, in_=tp_x[0:96, 1024:2048])
                csT = xts.tile([96, 1024], f32, tag="csT")
                nc.vector.tensor_copy(out=csT, in_=tp_c[0:96, 0:1024])

                # repack x per step: [12 feats, (A|B half, 512)] -> stacked
                # [24, 512] rows ordered (f0A,f0B,f1A,f1B,...) via SBUF->SBUF DMA
                xt_all = xts.tile([2 * F, T * NT], bf16, tag="xt_all")
                for t in range(T):
                    src = (xT_lo[12 * t:12 * t + 12, :] if t < 8
                           else xT_hi[12 * t - 84:12 * t - 72, :])
                    nc.sync.dma_start(
                        out=xt_all[:, t * NT:(t + 1) * NT],
                        in_=src.rearrange("p (h x) -> p h x", h=2))

                # ---------- initial cell states ----------
                ip = pp_g0.tile([128, 2048], f32, tag="G0")
                nc.tensor.matmul(ip[:, 0:512], fc1A, csT[:, 0:512],
                                 start=True, stop=False)
                nc.tensor.matmul(ip[:, 0:512], fc1B, csT[:, 512:1024],
                                 start=False, stop=True)
                nc.tensor.matmul(ip[:, 512:1024], fc2, csT[:, 0:512],
                                 start=True, stop=True)
                nc.tensor.matmul(ip[:, 1024:1536], fc2, csT[:, 512:1024],
                                 start=True, stop=True)
                c0 = states.tile([128, NT], f32, tag="c0")
                c1A = states.tile([H2, NT], f32, tag="c1A")
                c1B = states.tile([H2, NT], f32, tag="c1B")
                if has_vec_bias:
                    nc.vector.tensor_scalar_add(c0, ip[:, 0:512], vbias[:, 0:1])
                    nc.vector.tensor_scalar_add(c1A, ip[:, 512:1024], vbias[:, 1:2])
                    nc.vector.tensor_scalar_add(c1B, ip[:, 1024:1536], vbias[:, 1:2])
                else:
                    nc.vector.tensor_copy(out=c0, in_=ip[:, 0:512])
                    nc.vector.tensor_copy(out=c1A, in_=ip[:, 512:1024])
                    nc.vector.tensor_copy(out=c1B, in_=ip[:, 1024:1536])

                h0 = None
                h1 = [None, None]
                c1 = [c1A, c1B]
                for t in range(T):
                    x_t = xt_all[:, t * NT:(t + 1) * NT]
                    # ---- L0 gates, both halves stacked; cols [i|f|o|g] ----
                    G0 = pp_g0.tile([128, 2048], f32, tag="G0")
                    for gi in range(4):
                        reg = G0[:, gi * 512:(gi + 1) * 512]
                        nc.tensor.matmul(reg, w0ih[:, gi * 128:(gi + 1) * 128],
                                         x_t, start=True, stop=(t == 0))
                        if t > 0:
                            nc.tensor.matmul(reg, w0hh[:, gi * 128:(gi + 1) * 128],
                                             h0, start=False, stop=True)
                    if has_gate_bias:
                        for gi in range(4):
                            nc.vector.tensor_scalar_add(
                                G0[:, gi * 512:(gi + 1) * 512],
                                G0[:, gi * 512:(gi + 1) * 512], gbias[:, gi:gi + 1])
                    sig0 = scratch.tile([128, 1536], bf16, tag="sig0")
                    nc.scalar.activation(sig0, G0[:, 0:1536], AF.Sigmoid)
                    g0t = scratch.tile([128, NT], bf16, tag="g0t")
                    nc.scalar.activation(g0t, G0[:, 1536:2048], AF.Tanh)
                    t1_0 = scratch.tile([128, NT], bf16, tag="t1_0")
                    nc.vector.tensor_mul(t1_0, sig0[:, 512:1024], c0)
                    t2_0 = scratch.tile([128, NT], bf16, tag="t2_0")
                    nc.vector.tensor_mul(t2_0, sig0[:, 0:512], g0t)
                    cres0 = scratch.tile([128, NT], bf16, tag="cres0")
                    nc.vector.tensor_add(cres0, t1_0, t2_0)
                    c0n = states.tile([128, NT], f32, tag="c0")
                    nc.vector.tensor_add(c0n, c0, cres0)
                    c0 = c0n
                    tc0 = scratch.tile([128, NT], bf16, tag="tc0")
                    nc.scalar.activation(tc0, cres0, AF.Tanh)
                    h0n = states.tile([128, NT], bf16, tag="h0")
                    nc.vector.tensor_mul(h0n, sig0[:, 1024:1536], tc0)
                    h0 = h0n

                    # ---- L1 per half; cols [i|f|o|g] ----
                    for hf in range(2):
                        w1ih = w1ihA if hf == 0 else w1ihB
                        G1 = pp_g1.tile([128, 2048], f32, tag="G1")
                        for ci in range(4):
                            reg = G1[:, ci * 512:(ci + 1) * 512]
                            nc.tensor.matmul(reg, w1ih[:, ci * 128:(ci + 1) * 128],
                                             h0, start=True, stop=(t == 0))
                            if t > 0:
                                nc.tensor.matmul(reg, w1hh[:, ci * 128:(ci + 1) * 128],
                                                 h1[hf], start=False, stop=True)
                        if has_gate_bias:
                            for ci in range(4):
                                nc.vector.tensor_scalar_add(
                                    G1[:, ci * 512:(ci + 1) * 512],
                                    G1[:, ci * 512:(ci + 1) * 512],
                                    gbias[:, 4 + ci:5 + ci])
                        sig1 = scratch.tile([128, 1536], bf16, tag="sig1")
                        nc.scalar.activation(sig1, G1[:, 0:1536], AF.Sigmoid)
                        g1t = scratch.tile([128, NT], bf16, tag="g1t")
                        nc.scalar.activation(g1t, G1[:, 1536:2048], AF.Tanh)
                        t1_1 = scratch.tile([128, NT], bf16, tag="t1_1")
                        nc.vector.tensor_mul(t1_1, sig1[:, 512:1024], c1[hf])
                        t2_1 = scratch.tile([128, NT], bf16, tag="t2_1")
                        nc.vector.tensor_mul(t2_1, sig1[:, 0:512], g1t)
                        cres1 = scratch.tile([128, NT], bf16, tag="cres1")
                        nc.vector.tensor_add(cres1, t1_1, t2_1)
                        c1n = states.tile([H2, NT], f32, tag=f"c1{'AB'[hf]}")
                        nc.vector.tensor_add(c1n, c1[hf], cres1)
                        c1[hf] = c1n
                        tc1 = scratch.tile([128, NT], bf16, tag="tc1")
                        nc.scalar.activation(tc1, cres1, AF.Tanh)
                        h1n = states.tile([H2, NT], bf16, tag=f"h1{'AB'[hf]}")
                        nc.vector.tensor_mul(h1n, sig1[:, 1024:1536], tc1)
                        h1[hf] = h1n

                # ---------- head: pred = (h1 @ d1.T + b) @ d2.T + b ----------
                hp = pp_g0.tile([128, 2048], f32, tag="G0")
                for hf in range(2):
                    nc.tensor.matmul(hp[0:H1, hf * 512:(hf + 1) * 512], d1w, h1[hf],
                                     start=True, stop=True)
                    z = outp.tile([H1, NT], bf16, tag="z")
                    if has_vec_bias:
                        nc.vector.tensor_scalar_add(
                            z, hp[0:H1, hf * 512:(hf + 1) * 512], vbias[0:H1, 2:3])
                    else:
                        nc.vector.tensor_copy(out=z, in_=hp[0:H1, hf * 512:(hf + 1) * 512])
                    nc.tensor.matmul(hp[0:1, 1024 + hf * 512:1024 + (hf + 1) * 512],
                                     d2w, z, start=True, stop=True)
                    out_sb = outp.tile([1, NT], f32, tag="out_sb")
                    if has_vec_bias:
                        nc.vector.tensor_scalar_add(
                            out_sb, hp[0:1, 1024 + hf * 512:1024 + (hf + 1) * 512],
                            vbias[0:1, 3:4])
                    else:
                        nc.vector.tensor_copy(
                            out=out_sb, in_=hp[0:1, 1024 + hf * 512:1024 + (hf + 1) * 512])
                    nc.sync.dma_start(out=pred_view[it][hf], in_=out_sb)

            with tc.For_i(0, NPAIR, 1,
                          hint_engines=(nc.tensor.engine, nc.vector.engine,
                                        nc.scalar.engine)) as it:
                pair_body(it)

    nc.finalize()
    return nc


def _get_nc(key):
    if key not in _BUILD_CACHE:
        _BUILD_CACHE[key] = _build_bass(*key)
    return _BUILD_CACHE[key]


def _prep_weights(inputs):
    # gate order permutation i,f,g,o -> i,f,o,g (sigmoid gates contiguous)
    def perm(n):
        return np.concatenate([np.arange(0, 2 * n), np.arange(3 * n, 4 * n),
                               np.arange(2 * n, 3 * n)])
    p0, p1 = perm(H1), perm(H2)

    w0ihT = inputs["l0_w_ih"][p0].T.astype(np.float32)     # [12, 256]
    w0hhT = inputs["l0_w_hh"][p0].T.astype(np.float32)     # [64, 256]
    w1ihT = inputs["l1_w_ih"][p1].T.astype(np.float32)     # [64, 512]
    w1hhT = inputs["l1_w_hh"][p1].T.astype(np.float32)     # [128, 512]

    # L0 ih block-diagonal, rows interleaved (f,half) to match the repack DMA
    w0ih_bd = np.zeros((2 * F, 512), np.float32)
    w0ih_bd[0::2, :] = np.concatenate(
        [np.pad(w0ihT[:, g * 64:(g + 1) * 64], [(0, 0), (0, 64)])
         for g in range(4)], axis=1)                       # A rows -> cols 0:64 of each gate
    w0ih_bd[1::2, :] = np.concatenate(
        [np.pad(w0ihT[:, g * 64:(g + 1) * 64], [(0, 0), (64, 0)])
         for g in range(4)], axis=1)                       # B rows -> cols 64:128
    # L0 hh block-diagonal (A rows 0:64, B rows 64:128)
    w0hh_bd = np.zeros((2 * H1, 512), np.float32)
    for g in range(4):
        blk = w0hhT[:, g * 64:(g + 1) * 64]
        w0hh_bd[0:64, g * 128:g * 128 + 64] = blk
        w0hh_bd[64:128, g * 128 + 64:(g + 1) * 128] = blk
    # L1 ih half-masked (reads stacked h0)
    w1ih_A = np.concatenate([w1ihT, np.zeros_like(w1ihT)], axis=0)   # [128, 512]
    w1ih_B = np.concatenate([np.zeros_like(w1ihT), w1ihT], axis=0)
    fc1T = inputs["fc1_w"].T.astype(np.float32)            # [96, 64]
    fc1_A = np.concatenate([fc1T, np.zeros_like(fc1T)], axis=1)      # [96, 128]
    fc1_B = np.concatenate([np.zeros_like(fc1T), fc1T], axis=1)

    wm = {
        "w0ih_bd": w0ih_bd.astype(BF16),
        "w0hh_bd": w0hh_bd.astype(BF16),
        "w1ih_A": w1ih_A.astype(BF16),
        "w1ih_B": w1ih_B.astype(BF16),
        "w1hhT": np.ascontiguousarray(w1hhT).astype(BF16),
        "fc1_A": fc1_A,
        "fc1_B": fc1_B,
        "fc2T": np.ascontiguousarray(inputs["fc2_w"].T).astype(np.float32),
        "d1T": np.ascontiguousarray(inputs["d1_w"].T).astype(BF16),
        "d2T": np.ascontiguousarray(inputs["d2_w"].T).astype(BF16),
    }

    b0 = (inputs["l0_b_ih"] + inputs["l0_b_hh"]).astype(np.float32)[p0]   # [256]
    b1 = (inputs["l1_b_ih"] + inputs["l1_b_hh"]).astype(np.float32)[p1]   # [512]
    gb = np.zeros((128, 8), np.float32)
    for g in range(4):
        gb[:, g] = np.tile(b0[g * 64:(g + 1) * 64], 2)     # stacked [A;B]
        gb[:, 4 + g] = b1[g * 128:(g + 1) * 128]
    vb = np.zeros((128, 4), np.float32)
    vb[:, 0] = np.tile(inputs["fc1_b"], 2)
    vb[:, 1] = inputs["fc2_b"]
    vb[0:H1, 2] = inputs["d1_b"]
    vb[0:1, 3] = inputs["d2_b"]
    wm["gate_bias"] = gb
    wm["vec_bias"] = vb
    has_gate_bias = bool(np.any(b0) or np.any(b1))
    has_vec_bias = bool(np.any(vb))
    return wm, has_gate_bias, has_vec_bias


def _in_maps(inputs, wm):
    x = inputs["input_seq"].astype(np.float32, copy=False)
    cs = inputs["cell_state"].astype(np.float32, copy=False)
    maps = []
    for i in range(NCORES):
        m = dict(wm)
        m["input_seq"] = np.ascontiguousarray(x[i * BL:(i + 1) * BL])
        m["cell_state"] = np.ascontiguousarray(cs[i * BL:(i + 1) * BL])
        maps.append(m)
    return maps


def kernel(**inputs):
    inputs = {k: np.asarray(v) for k, v in inputs.items()}
    wm, hgb, hvb = _prep_weights(inputs)
    nc = _get_nc((hgb, hvb))
    from concourse.bass_utils import run_bass_kernel_spmd
    res = run_bass_kernel_spmd(nc, _in_maps(inputs, wm),
                               core_ids=list(range(NCORES)))
    return np.concatenate([r["pred"] for r in res.results], axis=0)


# revision 6
# speedup vs baseline: 10.2930x; 10.2930x over previous
# Trainium2 Bass kernel for nn_CauRecNet (2-layer residual-cell LSTM scan).
#
# Strategy: pure data-parallel over 8 NeuronCores (batch 131072 -> 16384/core).
# Per core: For_i over 16 "pair tiles" of 1024 batch rows = two 512-halves
# (A, B). Activations are feature-major ([feature, batch] in SBUF) so batch
# rides the matmul free dim (N=512).
#
# The small layer-0 (H1=64) is processed with both halves STACKED on the
# partition axis ([A;B] -> 128 rows) so every pointwise op runs on full
# [128,512] tiles. The stacked gate matmuls use BLOCK-DIAGONAL weights
# (lhsT = [[W,0],[0,W]]) so they stay plain 128x128-mode matmuls.
# Layer-1 (H2=128) runs per half; its ih-matmul reads the stacked h0 via
# half-masked (zero-padded) weights.
#
# dtypes: matmuls bf16, gate activations / intermediates bf16, c-state fp32.
# numpy-simulated rel-l2 error vs fp32 reference: ~5e-3.

import numpy as np
import ml_dtypes

B, T, F = 131072, 15, 12
H1, H2, CS = 64, 128, 96
NCORES = 8
BL = B // NCORES          # 16384 rows per core
NT = 512                  # matmul free dim (one half)
NPAIR = BL // (2 * NT)    # 16 pair-tiles per core

BF16 = ml_dtypes.bfloat16

_BUILD_CACHE = {}


def _build_bass(has_gate_bias, has_vec_bias, repeat=1):
    import concourse.bacc as bacc
    import concourse.tile as tile
    from concourse import mybir
    from concourse.masks import make_identity

    f32 = mybir.dt.float32
    bf16 = mybir.dt.bfloat16
    AF = mybir.ActivationFunctionType

    nc = bacc.Bacc()

    # ---- DRAM I/O ----
    x_d = nc.dram_tensor("input_seq", [BL, T, F], f32, kind="ExternalInput")
    cs_d = nc.dram_tensor("cell_state", [BL, CS], f32, kind="ExternalInput")
    w0ih_d = nc.dram_tensor("w0ih_bd", [2 * F, 4 * H1 * 2], bf16, kind="ExternalInput")
    w0hh_d = nc.dram_tensor("w0hh_bd", [2 * H1, 4 * H1 * 2], bf16, kind="ExternalInput")
    w1ihA_d = nc.dram_tensor("w1ih_A", [2 * H1, 4 * H2], bf16, kind="ExternalInput")
    w1ihB_d = nc.dram_tensor("w1ih_B", [2 * H1, 4 * H2], bf16, kind="ExternalInput")
    w1hh_d = nc.dram_tensor("w1hhT", [H2, 4 * H2], bf16, kind="ExternalInput")
    fc1A_d = nc.dram_tensor("fc1_A", [CS, 2 * H1], f32, kind="ExternalInput")
    fc1B_d = nc.dram_tensor("fc1_B", [CS, 2 * H1], f32, kind="ExternalInput")
    fc2_d = nc.dram_tensor("fc2T", [CS, H2], f32, kind="ExternalInput")
    d1_d = nc.dram_tensor("d1T", [H2, H1], bf16, kind="ExternalInput")
    d2_d = nc.dram_tensor("d2T", [H1, 1], bf16, kind="ExternalInput")
    gb_d = nc.dram_tensor("gate_bias", [128, 8], f32, kind="ExternalInput")
    vb_d = nc.dram_tensor("vec_bias", [128, 4], f32, kind="ExternalInput")
    pred_d = nc.dram_tensor("pred", [BL, 1], f32, kind="ExternalOutput")

    # DRAM views ([pair, ...])
    x_view = x_d[:].rearrange("(n c p) t f -> n p c (t f)", c=8, p=128)    # [16,128,8,180]
    cs_view = cs_d[:].rearrange("(n c p) k -> n p c k", c=8, p=128)        # [16,128,8,96]
    pred_view = pred_d[:].rearrange("(n h x) o -> n h o x", h=2, x=NT)     # [16,2,1,512]

    with tile.TileContext(nc) as tc:
        import contextlib
        ctx = contextlib.ExitStack()
        with ctx:
            consts = ctx.enter_context(tc.tile_pool(name="consts", bufs=1))
            loads = ctx.enter_context(tc.tile_pool(name="loads", bufs=2))
            xts = ctx.enter_context(tc.tile_pool(name="xts", bufs=2))
            states = ctx.enter_context(tc.tile_pool(name="states", bufs=3))
            scratch = ctx.enter_context(tc.tile_pool(name="scratch", bufs=2))
            outp = ctx.enter_context(tc.tile_pool(name="outp", bufs=2))
            pp_g0 = ctx.enter_context(tc.tile_pool(name="pp_g0", bufs=1, space="PSUM"))
            pp_g1 = ctx.enter_context(tc.tile_pool(name="pp_g1", bufs=1, space="PSUM"))

            # ---- constants / weights (loaded once) ----
            ident = consts.tile([128, 128], f32)
            make_identity(nc, ident)

            def load_const(name, dram, shape, dt):
                t = consts.tile(shape, dt, name=name)
                nc.sync.dma_start(out=t, in_=dram[:])
                return t

            w0ih = load_const("w0ih", w0ih_d, [2 * F, 512], bf16)
            w0hh = load_const("w0hh", w0hh_d, [2 * H1, 512], bf16)
            w1ihA = load_const("w1ihA", w1ihA_d, [2 * H1, 512], bf16)
            w1ihB = load_const("w1ihB", w1ihB_d, [2 * H1, 512], bf16)
            w1hh = load_const("w1hh", w1hh_d, [H2, 512], bf16)
            fc1A = load_const("fc1A", fc1A_d, [CS, 128], f32)
            fc1B = load_const("fc1B", fc1B_d, [CS, 128], f32)
            fc2 = load_const("fc2", fc2_d, [CS, H2], f32)
            d1w = load_const("d1w", d1_d, [H2, H1], bf16)
            d2w = load_const("d2w", d2_d, [H1, 1], bf16)
            gbias = load_const("gbias", gb_d, [128, 8], f32)
            vbias = load_const("vbias", vb_d, [128, 4], f32)

            def pair_body(it):
                # ---------- load ----------
                x_nat = loads.tile([128, 8, T * F], f32, tag="x_nat")
                nc.sync.dma_start(out=x_nat, in_=x_view[it])
                cs_nat = loads.tile([128, 8, CS], f32, tag="cs_nat")
                nc.sync.dma_start(out=cs_nat, in_=cs_view[it])

                # ---------- transpose to feature-major (PE) ----------
                tp_x = pp_g1.tile([128, 2048], f32, tag="G1")
                tp_c = pp_g1.tile([128, 2048], f32, tag="G1")
                for c in range(8):
                    nc.tensor.transpose(tp_x[0:96, c * 128:(c + 1) * 128],
                                        x_nat[:, c, 0:96], ident)
                    nc.tensor.transpose(tp_x[0:96, 1024 + c * 128:1024 + (c + 1) * 128],
                                        x_nat[:, c, 84:180], ident)
                    nc.tensor.transpose(tp_c[0:96, c * 128:(c + 1) * 128],
                                        cs_nat[:, c, :], ident)
                xT_lo = xts.tile([96, 1024], bf16, tag="xT_lo")  # feats 0:96, t 0..7
                nc.vector.tensor_copy(out=xT_lo, in_=tp_x[0:96, 0:1024])
                xT_hi = xts.tile([96, 1024], bf16, tag="xT_hi")  # feats 84:180, t 7..14
                nc.vector.tensor_copy(out=xT_hi, in_=tp_x[0:96, 1024:2048])
                csT = xts.tile([96, 1024], f32, tag="csT")
                nc.vector.tensor_copy(out=csT, in_=tp_c[0:96, 0:1024])

                # repack x per step: [12 feats, (A|B half, 512)] -> stacked
                # [24, 512] rows ordered (f0A,f0B,f1A,f1B,...) via SBUF->SBUF DMA
                xt_all = xts.tile([2 * F, T * NT], bf16, tag="xt_all")
                for t in range(T):
                    src = (xT_lo[12 * t:12 * t + 12, :] if t < 8
                           else xT_hi[12 * t - 84:12 * t - 72, :])
                    nc.sync.dma_start(
                        out=xt_all[:, t * NT:(t + 1) * NT],
                        in_=src.rearrange("p (h x) -> p h x", h=2))

                # ---------- initial cell states ----------
                ip = pp_g0.tile([128, 2048], f32, tag="G0")
                nc.tensor.matmul(ip[:, 0:512], fc1A, csT[:, 0:512],
                                 start=True, stop=False)
                nc.tensor.matmul(ip[:, 0:512], fc1B, csT[:, 512:1024],
                                 start=False, stop=True)
                nc.tensor.matmul(ip[:, 512:1024], fc2, csT[:, 0:512],
                                 start=True, stop=True)
                nc.tensor.matmul(ip[:, 1024:1536], fc2, csT[:, 512:1024],
                                 start=True, stop=True)
                c0 = states.tile([128, NT], f32, tag="c0")
                c1A = states.tile([H2, NT], f32, tag="c1A")
                c1B = states.tile([H2, NT], f32, tag="c1B")
                if has_vec_bias:
                    nc.vector.tensor_scalar_add(c0, ip[:, 0:512], vbias[:, 0:1])
                    nc.vector.tensor_scalar_add(c1A, ip[:, 512:1024], vbias[:, 1:2])
                    nc.vector.tensor_scalar_add(c1B, ip[:, 1024:1536], vbias[:, 1:2])
                else:
                    nc.vector.tensor_copy(out=c0, in_=ip[:, 0:512])
                    nc.vector.tensor_copy(out=c1A, in_=ip[:, 512:1024])
                    nc.vector.tensor_copy(out=c1B, in_=ip[:, 1024:1536])

                h0 = None
                h1 = [None, None]
                c1 = [c1A, c1B]
                for t in range(T):
                    x_t = xt_all[:, t * NT:(t + 1) * NT]
                    # ---- L0 gates, both halves stacked; cols [i|f|o|g] ----
                    G0 = pp_g0.tile([128, 2048], f32, tag="G0")
                    for gi in range(4):
                        reg = G0[:, gi * 512:(gi + 1) * 512]
                        nc.tensor.matmul(reg, w0ih[:, gi * 128:(gi + 1) * 128],
                                         x_t, start=True, stop=(t == 0))
                        if t > 0:
                            nc.tensor.matmul(reg, w0hh[:, gi * 128:(gi + 1) * 128],
                                             h0, start=False, stop=True)
                    if has_gate_bias:
                        for gi in range(4):
                            nc.vector.tensor_scalar_add(
                                G0[:, gi * 512:(gi + 1) * 512],
                                G0[:, gi * 512:(gi + 1) * 512], gbias[:, gi:gi + 1])
                    sig0 = scratch.tile([128, 1536], bf16, tag="sig0")
                    nc.scalar.activation(sig0, G0[:, 0:1536], AF.Sigmoid)
                    g0t = scratch.tile([128, NT], bf16, tag="g0t")
                    nc.scalar.activation(g0t, G0[:, 1536:2048], AF.Tanh)
                    t1_0 = scratch.tile([128, NT], bf16, tag="t1_0")
                    nc.vector.tensor_mul(t1_0, sig0[:, 512:1024], c0)
                    t2_0 = scratch.tile([128, NT], bf16, tag="t2_0")
                    nc.vector.tensor_mul(t2_0, sig0[:, 0:512], g0t)
                    cres0 = scratch.tile([128, NT], bf16, tag="cres0")
                    nc.vector.tensor_add(cres0, t1_0, t2_0)
                    c0n = states.tile([128, NT], f32, tag="c0")
                    nc.vector.tensor_add(c0n, c0, cres0)
                    c0 = c0n
                    tc0 = scratch.tile([128, NT], bf16, tag="tc0")
                    nc.scalar.activation(tc0, cres0, AF.Tanh)
                    h0n = states.tile([128, NT], bf16, tag="h0")
                    nc.vector.tensor_mul(h0n, sig0[:, 1024:1536], tc0)
                    h0 = h0n

                    # ---- L1 per half; cols [i|f|o|g] ----
                    for hf in range(2):
                        w1ih = w1ihA if hf == 0 else w1ihB
                        G1 = pp_g1.tile([128, 2048], f32, tag="G1")
                        for ci in range(4):
                            reg = G1[:, ci * 512:(ci + 1) * 512]
                            nc.tensor.matmul(reg, w1ih[:, ci * 128:(ci + 1) * 128],
                                             h0, start=True, stop=(t == 0))
                            if t > 0:
                                nc.tensor.matmul(reg, w1hh[:, ci * 128:(ci + 1) * 128],
                                                 h1[hf], start=False, stop=True)
                        if has_gate_bias:
                            for ci in range(4):
                                nc.vector.tensor_scalar_add(
                                    G1[:, ci * 512:(ci + 1) * 512],
                                    G1[:, ci * 512:(ci + 1) * 512],
                                    gbias[:, 4 + ci:5 + ci])
                        sig1 = scratch.tile([128, 1536], bf16, tag="sig1")
                        nc.scalar.activation(sig1, G1[:, 0:1536], AF.Sigmoid)
                        g1t = scratch.tile([128, NT], bf16, tag="g1t")
                        nc.scalar.activation(g1t, G1[:, 1536:2048], AF.Tanh)
                        t1_1 = scratch.tile([128, NT], bf16, tag="t1_1")
                        nc.vector.tensor_mul(t1_1, sig1[:, 512:1024], c1[hf])
                        t2_1 = scratch.tile([128, NT], bf16, tag="t2_1")
                        nc.vector.tensor_mul(t2_1, sig1[:, 0:512], g1t)
                        cres1 = scratch.tile([128, NT], bf16, tag="cres1")
                        nc.vector.tensor_add(cres1, t1_1, t2_1)
                        c1n = states.tile([H2, NT], f32, tag=f"c1{'AB'[hf]}")
                        nc.vector.tensor_add(c1n, c1[hf], cres1)
                        c1[hf] = c1n
                        tc1 = scratch.tile([128, NT], bf16, tag="tc1")
                        nc.scalar.activation(tc1, cres1, AF.Tanh)
                        h1n = states.tile([H2, NT], bf16, tag=f"h1{'AB'[hf]}")
                        nc.vector.tensor_mul(h1n, sig1[:, 1024:1536], tc1)
                        h1[hf] = h1n

                # ---------- head: pred = (h1 @ d1.T + b) @ d2.T + b ----------
                hp = pp_g0.tile([128, 2048], f32, tag="G0")
                for hf in range(2):
                    nc.tensor.matmul(hp[0:H1, hf * 512:(hf + 1) * 512], d1w, h1[hf],
                                     start=True, stop=True)
                    z = outp.tile([H1, NT], bf16, tag="z")
                    if has_vec_bias:
                        nc.vector.tensor_scalar_add(
                            z, hp[0:H1, hf * 512:(hf + 1) * 512], vbias[0:H1, 2:3])
                    else:
                        nc.vector.tensor_copy(out=z, in_=hp[0:H1, hf * 512:(hf + 1) * 512])
                    nc.tensor.matmul(hp[0:1, 1024 + hf * 512:1024 + (hf + 1) * 512],
                                     d2w, z, start=True, stop=True)
                    out_sb = outp.tile([1, NT], f32, tag="out_sb")
                    if has_vec_bias:
                        nc.vector.tensor_scalar_add(
                            out_sb, hp[0:1, 1024 + hf * 512:1024 + (hf + 1) * 512],
                            vbias[0:1, 3:4])
                    else:
                        nc.vector.tensor_copy(
                            out=out_sb, in_=hp[0:1, 1024 + hf * 512:1024 + (hf + 1) * 512])
                    nc.sync.dma_start(out=pred_view[it][hf], in_=out_sb)

            if repeat == 1:
                with tc.For_i(0, NPAIR, 1,
                              hint_engines=(nc.tensor.engine, nc.vector.engine,
                                            nc.scalar.engine)) as it:
                    pair_body(it)
            else:  # benchmark variant: run the whole workload `repeat` times
                with tc.For_i(0, repeat, 1) as _r:
                    with tc.For_i(0, NPAIR, 1,
                                  hint_engines=(nc.tensor.engine, nc.vector.engine,
                                                nc.scalar.engine)) as it:
                        pair_body(it)

    nc.finalize()
    return nc


def _get_nc(key):
    if key not in _BUILD_CACHE:
        _BUILD_CACHE[key] = _build_bass(*key)
    return _BUILD_CACHE[key]


def _prep_weights(inputs):
    # gate order permutation i,f,g,o -> i,f,o,g (sigmoid gates contiguous)
    def perm(n):
        return np.concatenate([np.arange(0, 2 * n), np.arange(3 * n, 4 * n),
                               np.arange(2 * n, 3 * n)])
    p0, p1 = perm(H1), perm(H2)

    w0ihT = inputs["l0_w_ih"][p0].T.astype(np.float32)     # [12, 256]
    w0hhT = inputs["l0_w_hh"][p0].T.astype(np.float32)     # [64, 256]
    w1ihT = inputs["l1_w_ih"][p1].T.astype(np.float32)     # [64, 512]
    w1hhT = inputs["l1_w_hh"][p1].T.astype(np.float32)     # [128, 512]

    # L0 ih block-diagonal, rows interleaved (f,half) to match the repack DMA
    w0ih_bd = np.zeros((2 * F, 512), np.float32)
    w0ih_bd[0::2, :] = np.concatenate(
        [np.pad(w0ihT[:, g * 64:(g + 1) * 64], [(0, 0), (0, 64)])
         for g in range(4)], axis=1)                       # A rows -> cols 0:64 of each gate
    w0ih_bd[1::2, :] = np.concatenate(
        [np.pad(w0ihT[:, g * 64:(g + 1) * 64], [(0, 0), (64, 0)])
         for g in range(4)], axis=1)                       # B rows -> cols 64:128
    # L0 hh block-diagonal (A rows 0:64, B rows 64:128)
    w0hh_bd = np.zeros((2 * H1, 512), np.float32)
    for g in range(4):
        blk = w0hhT[:, g * 64:(g + 1) * 64]
        w0hh_bd[0:64, g * 128:g * 128 + 64] = blk
        w0hh_bd[64:128, g * 128 + 64:(g + 1) * 128] = blk
    # L1 ih half-masked (reads stacked h0)
    w1ih_A = np.concatenate([w1ihT, np.zeros_like(w1ihT)], axis=0)   # [128, 512]
    w1ih_B = np.concatenate([np.zeros_like(w1ihT), w1ihT], axis=0)
    fc1T = inputs["fc1_w"].T.astype(np.float32)            # [96, 64]
    fc1_A = np.concatenate([fc1T, np.zeros_like(fc1T)], axis=1)      # [96, 128]
    fc1_B = np.concatenate([np.zeros_like(fc1T), fc1T], axis=1)

    wm = {
        "w0ih_bd": w0ih_bd.astype(BF16),
        "w0hh_bd": w0hh_bd.astype(BF16),
        "w1ih_A": w1ih_A.astype(BF16),
        "w1ih_B": w1ih_B.astype(BF16),
        "w1hhT": np.ascontiguousarray(w1hhT).astype(BF16),
        "fc1_A": fc1_A,
        "fc1_B": fc1_B,
        "fc2T": np.ascontiguousarray(inputs["fc2_w"].T).astype(np.float32),
        "d1T": np.ascontiguousarray(inputs["d1_w"].T).astype(BF16),
        "d2T": np.ascontiguousarray(inputs["d2_w"].T).astype(BF16),
    }

    b0 = (inputs["l0_b_ih"] + inputs["l0_b_hh"]).astype(np.float32)[p0]   # [256]
    b1 = (inputs["l1_b_ih"] + inputs["l1_b_hh"]).astype(np.float32)[p1]   # [512]
    gb = np.zeros((128, 8), np.float32)
    for g in range(4):
        gb[:, g] = np.tile(b0[g * 64:(g + 1) * 64], 2)     # stacked [A;B]
        gb[:, 4 + g] = b1[g * 128:(g + 1) * 128]
    vb = np.zeros((128, 4), np.float32)
    vb[:, 0] = np.tile(inputs["fc1_b"], 2)
    vb[:, 1] = inputs["fc2_b"]
    vb[0:H1, 2] = inputs["d1_b"]
    vb[0:1, 3] = inputs["d2_b"]
    wm["gate_bias"] = gb
    wm["vec_bias"] = vb
    has_gate_bias = bool(np.any(b0) or np.any(b1))
    has_vec_bias = bool(np.any(vb))
    return wm, has_gate_bias, has_vec_bias


def _in_maps(inputs, wm):
    x = inputs["input_seq"].astype(np.float32, copy=False)
    cs = inputs["cell_state"].astype(np.float32, copy=False)
    maps = []
    for i in range(NCORES):
        m = dict(wm)
        m["input_seq"] = np.ascontiguousarray(x[i * BL:(i + 1) * BL])
        m["cell_state"] = np.ascontiguousarray(cs[i * BL:(i + 1) * BL])
        maps.append(m)
    return maps


def kernel(**inputs):
    inputs = {k: np.asarray(v) for k, v in inputs.items()}
    wm, hgb, hvb = _prep_weights(inputs)
    nc = _get_nc((hgb, hvb))
    from concourse.bass_utils import run_bass_kernel_spmd
    res = run_bass_kernel_spmd(nc, _in_maps(inputs, wm),
                               core_ids=list(range(NCORES)))
    return np.concatenate([r["pred"] for r in res.results], axis=0)
